# revision 11
# baseline (speedup 1.0000x reference)
"""Trainium2 Bass kernel for ControlLoRACrossAttnProcessor.

Head-parallel sharding over 8 NeuronCores: core c owns attention head c
(columns c*128:(c+1)*128 of Wq/Wk/Wv, rows of the same range in Wo's
contraction dim). Each core computes its head's full attention and a
partial output projection; the host sums the 8 partials. The rank-4
LoRA path is sharded by sequence rows (core c owns rows c*512:(c+1)*512)
and returned as a separate small output that the host adds in, together
with the output bias (added exactly once per row).

All matmuls run as float32r (TF32-like PE mode, full rate at moving
dim >= 256) with fp32 accumulation in PSUM. Attention uses the
transposed-scores layout: scores^T tiles [k=128, q=512] so softmax's
exp rides ScalarE and the k-sums ride TensorE (ones-vector matmul);
normalization is deferred to the output projection (divide commutes
with the linear Wo projection).
"""

import sys
import types

for _p in ("/opt/trn_rl_repo", "/root/.axon_site"):
    if _p not in sys.path:
        sys.path.insert(0, _p)

import numpy as np

import concourse.bass as bass  # noqa: E402
import concourse.mybir as mybir  # noqa: E402
from concourse import bacc  # noqa: E402
from concourse.bass_utils import run_bass_kernel_spmd  # noqa: E402
from concourse.tile import TileContext  # noqa: E402
from concourse.masks import make_identity  # noqa: E402

dt = mybir.dt

B, S, D = 2, 2048, 1024
H = 8
HD = 128
RANK = 4
N_CORES = 8
SG = B * S            # 4096 flattened rows
ROWS_PER_CORE = SG // N_CORES  # 512
NSTRIP = S // 512     # 4 query strips of 512 per batch
NKT = S // 128        # 16 key tiles of 128 per batch
NQT = 512 // 128      # 4 query tiles per strip
INV_SQRT_HD = 1.0 / np.sqrt(np.float32(HD))

F32 = dt.float32
F32R = dt.float32r

_CACHE = {}


def build_program():
    if "nc" in _CACHE:
        return _CACHE["nc"]

    nc = bacc.Bacc("TRN2", target_bir_lowering=False, debug=False,
                   num_devices=N_CORES)

    xT = nc.declare_dram_parameter("xT", [D, SG], F32R, isOutput=False)
    wqT = nc.declare_dram_parameter("wqT", [D, HD], F32R, isOutput=False)
    wkT = nc.declare_dram_parameter("wkT", [D, HD], F32R, isOutput=False)
    wvT = nc.declare_dram_parameter("wvT", [D, HD], F32R, isOutput=False)
    woT = nc.declare_dram_parameter("woT", [HD, D], F32R, isOutput=False)
    cT = nc.declare_dram_parameter("cT", [D, ROWS_PER_CORE], F32R, isOutput=False)
    ldT = nc.declare_dram_parameter("ldT", [D, RANK], F32R, isOutput=False)
    luT = nc.declare_dram_parameter("luT", [RANK, D], F32R, isOutput=False)
    bo = nc.declare_dram_parameter("bo", [1, D], F32, isOutput=False)
    out = nc.declare_dram_parameter("out", [SG, D], F32, isOutput=True)
    lora_out = nc.declare_dram_parameter("lora_out", [ROWS_PER_CORE, D], F32,
                                         isOutput=True)

    with TileContext(nc) as tc:
        with tc.tile_pool(name="const", bufs=1) as constp, \
             tc.tile_pool(name="wts", bufs=1) as wts, \
             tc.tile_pool(name="mm_ps", bufs=3, space="PSUM") as mm_ps, \
             tc.tile_pool(name="sc_ps", bufs=2, space="PSUM") as sc_ps, \
             tc.tile_pool(name="at_ps", bufs=2, space="PSUM") as at_ps, \
             tc.tile_pool(name="sum_ps", bufs=1, space="PSUM") as sum_ps, \
             tc.tile_pool(name="xt", bufs=2) as xtp, \
             tc.tile_pool(name="qkv", bufs=2) as qkvp, \
             tc.tile_pool(name="es", bufs=8) as esp, \
             tc.tile_pool(name="small", bufs=2) as smallp, \
             tc.tile_pool(name="outp", bufs=3) as outp:

            # ---- constants & weights ----
            ident = constp.tile([128, 128], F32, tag="ident")
            make_identity(nc, ident[:])
            ones_f = constp.tile([128, 1], F32, tag="onesf")
            nc.vector.memset(ones_f[:], 1.0)
            ones = constp.tile([128, 1], F32R, tag="ones")
            nc.vector.tensor_copy(ones[:], ones_f[:])

            wq_sb = wts.tile([128, D], F32R, tag="wq")
            wk_sb = wts.tile([128, D], F32R, tag="wk")
            wv_sb = wts.tile([128, D], F32R, tag="wv")
            wo_sb = wts.tile([HD, D], F32R, tag="wo")
            lu_sb = wts.tile([RANK, D], F32R, tag="lu")
            ld_sb = wts.tile([128, 8 * RANK], F32R, tag="ld")
            bo_sb = wts.tile([1, D], F32, tag="bo")
            bo_bc = wts.tile([128, D], F32, tag="bobc")
            for _w_sb, _wT in ((wq_sb, wqT), (wk_sb, wkT), (wv_sb, wvT)):
                nc.sync.dma_start(
                    out=_w_sb[:].rearrange("p (t m) -> p t m", t=8),
                    in_=_wT[:].rearrange("(t p) m -> p t m", p=128))
            nc.sync.dma_start(out=wo_sb[:], in_=woT[:])
            nc.sync.dma_start(out=lu_sb[:], in_=luT[:])
            nc.sync.dma_start(out=ld_sb[:].rearrange("p (t m) -> p t m", t=8),
                              in_=ldT[:].rearrange("(t p) m -> p t m", p=128))
            nc.sync.dma_start(out=bo_sb[:], in_=bo[:])
            nc.gpsimd.partition_broadcast(bo_bc[:], bo_sb[:])

            # ---- LoRA path: rows [c*512, (c+1)*512) of ctrl + bias ----
            with tc.tile_pool(name="ct", bufs=1) as ctp:
                ct_sb = ctp.tile([128, 8 * ROWS_PER_CORE], F32R, tag="ct")
                nc.sync.dma_start(
                    out=ct_sb[:].rearrange("p (t m) -> p t m", t=8),
                    in_=cT[:].rearrange("(t p) m -> p t m", p=128))
                dn_ps = mm_ps.tile([RANK, ROWS_PER_CORE], F32, tag="mm")
                for d in range(8):
                    nc.tensor.matmul(
                        dn_ps[:],
                        (ld_sb[:, d * RANK:(d + 1) * RANK]),
                        (ct_sb[:, d * ROWS_PER_CORE:(d + 1) * ROWS_PER_CORE]),
                        start=(d == 0), stop=(d == 7))
                dn_sb = smallp.tile([RANK, ROWS_PER_CORE], F32R, tag="dn")
                nc.vector.tensor_copy(dn_sb[:], dn_ps[:])
                for j in range(ROWS_PER_CORE // 128):
                    lo_sb = outp.tile([128, D], F32, tag="osb")
                    for h in range(2):
                        up_ps = mm_ps.tile([128, 512], F32, tag="mm")
                        nc.tensor.matmul(
                            up_ps[:],
                            (dn_sb[:, j * 128:(j + 1) * 128]),
                            (lu_sb[:, h * 512:(h + 1) * 512]),
                            start=True, stop=True)
                        nc.vector.tensor_add(
                            lo_sb[:, h * 512:(h + 1) * 512], up_ps[:],
                            bo_bc[:, h * 512:(h + 1) * 512])
                    nc.sync.dma_start(out=lora_out[j * 128:(j + 1) * 128, :],
                                      in_=lo_sb[:])

            # ---- main per-batch attention ----
            for b in range(B):
                xt = [xtp.tile([128, 4 * S], F32R, tag="xt", name=f"xt{b}_{h}")
                      for h in range(2)]
                for h in range(2):
                    nc.sync.dma_start(
                        out=xt[h][:].rearrange("p (t s) -> p t s", t=4),
                        in_=xT[h * 512:(h + 1) * 512, b * S:(b + 1) * S]
                        .rearrange("(t p) s -> p t s", p=128))

                qt_sb = qkvp.tile([HD, S], F32R, tag="qt")
                kt_sb = qkvp.tile([HD, S], F32R, tag="kt")
                vt_sb = qkvp.tile([HD, S], F32, tag="vt", bufs=1)
                v_sb = qkvp.tile([128, S], F32R, tag="v")

                # Q^T, K^T, V^T: [hd, s] strips of 512, accumulate over 8 d-tiles
                for w_sb, dst in ((wq_sb, qt_sb), (wk_sb, kt_sb), (wv_sb, vt_sb)):
                    for strip in range(NSTRIP):
                        ps = mm_ps.tile([HD, 512], F32, tag="mm")
                        for h in range(2):
                            for dl in range(4):
                                d = h * 4 + dl
                                nc.tensor.matmul(
                                    ps[:],
                                    (w_sb[:, d * HD:(d + 1) * HD]),
                                    (xt[h][:, dl * S + strip * 512:
                                             dl * S + strip * 512 + 512]),
                                    start=(d == 0), stop=(d == 7))
                        nc.vector.tensor_copy(
                            dst[:, strip * 512:(strip + 1) * 512], ps[:])

                # V natural layout via PE transposes of V^T
                for kt in range(NKT):
                    tp = mm_ps.tile([128, 128], F32, tag="mm")
                    nc.tensor.transpose(
                        tp[:], vt_sb[:, kt * 128:(kt + 1) * 128], ident[:])
                    nc.vector.tensor_copy(v_sb[:, kt * 128:(kt + 1) * 128], tp[:])

                # attention strips
                for strip in range(NSTRIP):
                    q_sl = slice(strip * 512, (strip + 1) * 512)
                    at_ps_t = at_ps.tile([HD, 512], F32, tag="at")
                    sm_ps = sum_ps.tile([1, 512], F32, tag="sums")
                    es_tiles = []
                    for kt in range(NKT):
                        scp = sc_ps.tile([128, 512], F32, tag="sc")
                        nc.tensor.matmul(
                            scp[:],
                            (kt_sb[:, kt * 128:(kt + 1) * 128]),
                            (qt_sb[:, q_sl]),
                            start=True, stop=True)
                        es = esp.tile([128, 512], F32R, tag="es")
                        es_tiles.append(es)
                        nc.scalar.activation(
                            es[:], scp[:], mybir.ActivationFunctionType.Exp,
                            scale=float(INV_SQRT_HD))
                        nc.tensor.matmul(
                            at_ps_t[:],
                            (v_sb[:, kt * 128:(kt + 1) * 128]),
                            (es[:]),
                            start=(kt == 0), stop=(kt == NKT - 1),
                            skip_group_check=True)
                        nc.tensor.matmul(
                            sm_ps[:],
                            (ones[:]),
                            (es[:]),
                            start=(kt == 0), stop=(kt == NKT - 1),
                            skip_group_check=True)

                    # reciprocal of sums, scatter [1,512] row -> [128,4] cols
                    row_rc = smallp.tile([1, 512], F32, tag="rowrc")
                    nc.vector.reciprocal(row_rc[:], sm_ps[:])
                    rc_sb = smallp.tile([128, NQT], F32, tag="rc")
                    for j in range(NQT):
                        nc.sync.dma_start(
                            out=rc_sb[:, j:j + 1],
                            in_=row_rc[0:1, j * 128:(j + 1) * 128])

                    atn_sb = smallp.tile([HD, 512], F32R, tag="atn")
                    nc.vector.tensor_copy(atn_sb[:], at_ps_t[:])

                    # output projection + normalization
                    for j in range(NQT):
                        o_sb = outp.tile([128, D], F32, tag="osb")
                        for h in range(2):
                            op = mm_ps.tile([128, 512], F32, tag="mm")
                            nc.tensor.matmul(
                                op[:],
                                (atn_sb[:, j * 128:(j + 1) * 128]),
                                (wo_sb[:, h * 512:(h + 1) * 512]),
                                start=True, stop=True)
                            nc.vector.tensor_scalar_mul(
                                o_sb[:, h * 512:(h + 1) * 512], op[:],
                                rc_sb[:, j:j + 1])
                        r0 = b * S + strip * 512 + j * 128
                        nc.sync.dma_start(out=out[r0:r0 + 128, :], in_=o_sb[:])

    nc.compile()
    _CACHE["nc"] = nc
    return nc


def _prep_in_maps(inputs):
    hidden = np.ascontiguousarray(inputs["hidden_states"], dtype=np.float32)
    control = np.ascontiguousarray(inputs["control_states"], dtype=np.float32)
    Wq = np.asarray(inputs["Wq"], dtype=np.float32)
    Wk = np.asarray(inputs["Wk"], dtype=np.float32)
    Wv = np.asarray(inputs["Wv"], dtype=np.float32)
    Wo = np.asarray(inputs["Wo"], dtype=np.float32)
    bo = np.asarray(inputs["bo"], dtype=np.float32)
    ld = np.asarray(inputs["lora_down"], dtype=np.float32)
    lu = np.asarray(inputs["lora_up"], dtype=np.float32)

    xT = np.ascontiguousarray(hidden.reshape(SG, D).T)
    cT_full = np.ascontiguousarray(control.reshape(SG, D).T)
    ldT = np.ascontiguousarray(ld.T)
    luT = np.ascontiguousarray(lu.T)
    bo_in = np.ascontiguousarray(bo.reshape(1, D))

    in_maps = []
    for c in range(N_CORES):
        hs = slice(c * HD, (c + 1) * HD)
        rs = slice(c * ROWS_PER_CORE, (c + 1) * ROWS_PER_CORE)
        in_maps.append({
            "xT": xT,
            "wqT": np.ascontiguousarray(Wq[hs, :].T),
            "wkT": np.ascontiguousarray(Wk[hs, :].T),
            "wvT": np.ascontiguousarray(Wv[hs, :].T),
            "woT": np.ascontiguousarray(Wo[:, hs].T),
            "cT": np.ascontiguousarray(cT_full[:, rs]),
            "ldT": ldT,
            "luT": luT,
            "bo": bo_in,
        })
    return in_maps


def _reduce_outputs(results):
    total = np.zeros((SG, D), dtype=np.float64)
    for c in range(N_CORES):
        total += results[c]["out"].astype(np.float64)
    total = total.astype(np.float32)
    for c in range(N_CORES):
        rs = slice(c * ROWS_PER_CORE, (c + 1) * ROWS_PER_CORE)
        total[rs] += results[c]["lora_out"]
    return total.reshape(B, S, D)


def kernel(**inputs):
    nc = build_program()
    in_maps = _prep_in_maps(inputs)
    res = run_bass_kernel_spmd(nc, in_maps, list(range(N_CORES)))
    return _reduce_outputs(res.results)


# revision 18
# speedup vs baseline: 1.1623x; 1.1623x over previous
"""Trainium2 Bass kernel for ControlLoRACrossAttnProcessor.

Head-parallel sharding over 8 NeuronCores: core c owns attention head c
(columns c*128:(c+1)*128 of Wq/Wk/Wv, rows of the same range in Wo's
contraction dim). Each core computes its head's full attention and a
partial output projection; the host sums the 8 partials. The rank-4
LoRA path is sharded by sequence rows (core c owns rows c*512:(c+1)*512)
and returned as a separate small output that the host adds in, together
with the output bias (added exactly once per row).

All matmuls run as float32r (TF32-like PE mode, full rate at moving
dim >= 256) with fp32 accumulation in PSUM. Attention uses the
transposed-scores layout: scores^T tiles [k=128, q=512] so softmax's
exp rides ScalarE and the k-sums ride TensorE (ones-vector matmul);
normalization is deferred to the output projection (divide commutes
with the linear Wo projection).
"""

import sys
import types

for _p in ("/opt/trn_rl_repo", "/root/.axon_site"):
    if _p not in sys.path:
        sys.path.insert(0, _p)

import numpy as np

import concourse.bass as bass  # noqa: E402
import concourse.mybir as mybir  # noqa: E402
from concourse import bacc  # noqa: E402
from concourse.bass_utils import run_bass_kernel_spmd  # noqa: E402
from concourse.tile import TileContext  # noqa: E402
from concourse.masks import make_identity  # noqa: E402

dt = mybir.dt

B, S, D = 2, 2048, 1024
H = 8
HD = 128
RANK = 4
N_CORES = 8
SG = B * S            # 4096 flattened rows
ROWS_PER_CORE = SG // N_CORES  # 512
NSTRIP = S // 512     # 4 query strips of 512 per batch
NKT = S // 128        # 16 key tiles of 128 per batch
NQT = 512 // 128      # 4 query tiles per strip
INV_SQRT_HD = 1.0 / np.sqrt(np.float32(HD))

F32 = dt.float32
F32R = dt.float32r

_CACHE = {}


def build_program():
    if "nc" in _CACHE:
        return _CACHE["nc"]

    nc = bacc.Bacc("TRN2", target_bir_lowering=False, debug=False,
                   num_devices=N_CORES)

    xT = nc.declare_dram_parameter("xT", [D, SG], F32R, isOutput=False)
    wqT = nc.declare_dram_parameter("wqT", [D, HD], F32R, isOutput=False)
    wkT = nc.declare_dram_parameter("wkT", [D, HD], F32R, isOutput=False)
    wvT = nc.declare_dram_parameter("wvT", [D, HD], F32R, isOutput=False)
    woT = nc.declare_dram_parameter("woT", [HD, D], F32R, isOutput=False)
    cT = nc.declare_dram_parameter("cT", [D, ROWS_PER_CORE], F32R, isOutput=False)
    ldT = nc.declare_dram_parameter("ldT", [D, RANK], F32R, isOutput=False)
    luT = nc.declare_dram_parameter("luT", [RANK, D], F32R, isOutput=False)
    bo = nc.declare_dram_parameter("bo", [1, D], F32, isOutput=False)
    out = nc.declare_dram_parameter("out", [SG, D], F32, isOutput=True)
    lora_out = nc.declare_dram_parameter("lora_out", [ROWS_PER_CORE, D], F32,
                                         isOutput=True)

    with TileContext(nc) as tc:
        with tc.tile_pool(name="const", bufs=1) as constp, \
             tc.tile_pool(name="wts", bufs=1) as wts, \
             tc.tile_pool(name="mm_ps", bufs=3, space="PSUM") as mm_ps, \
             tc.tile_pool(name="sc_ps", bufs=3, space="PSUM") as sc_ps, \
             tc.tile_pool(name="at_ps", bufs=1, space="PSUM") as at_ps, \
             tc.tile_pool(name="sum_ps", bufs=1, space="PSUM") as sum_ps, \
             tc.tile_pool(name="xt", bufs=2) as xtp, \
             tc.tile_pool(name="qkv", bufs=2) as qkvp, \
             tc.tile_pool(name="es", bufs=8) as esp, \
             tc.tile_pool(name="small", bufs=2) as smallp, \
             tc.tile_pool(name="outp", bufs=3) as outp:

            # ---- constants & weights ----
            ident = constp.tile([128, 128], F32, tag="ident")
            make_identity(nc, ident[:])
            ones_f = constp.tile([128, 1], F32, tag="onesf")
            nc.vector.memset(ones_f[:], 1.0)
            ones = constp.tile([128, 1], F32R, tag="ones")
            nc.vector.tensor_copy(ones[:], ones_f[:])

            wq_sb = wts.tile([128, D], F32R, tag="wq")
            wk_sb = wts.tile([128, D], F32R, tag="wk")
            wv_sb = wts.tile([128, D], F32R, tag="wv")
            wo_sb = wts.tile([HD, D], F32R, tag="wo")
            lu_sb = wts.tile([RANK, D], F32R, tag="lu")
            ld_sb = wts.tile([128, 8 * RANK], F32R, tag="ld")
            bo_sb = wts.tile([1, D], F32, tag="bo")
            bo_bc = wts.tile([128, D], F32, tag="bobc")
            for _w_sb, _wT in ((wq_sb, wqT), (wk_sb, wkT), (wv_sb, wvT)):
                nc.sync.dma_start(
                    out=_w_sb[:].rearrange("p (t m) -> p t m", t=8),
                    in_=_wT[:].rearrange("(t p) m -> p t m", p=128))

            # batch-0 activations early so QKV can start ASAP
            xt_tiles = {}

            def load_xt(b):
                tiles = [xtp.tile([128, 4 * S], F32R, tag="xt",
                                  name=f"xt{b}_{h}") for h in range(2)]
                for h in range(2):
                    nc.sync.dma_start(
                        out=tiles[h][:].rearrange("p (t s) -> p t s", t=4),
                        in_=xT[h * 512:(h + 1) * 512, b * S:(b + 1) * S]
                        .rearrange("(t p) s -> p t s", p=128))
                xt_tiles[b] = tiles

            load_xt(0)

            nc.sync.dma_start(out=wo_sb[:], in_=woT[:])
            nc.sync.dma_start(out=lu_sb[:], in_=luT[:])
            nc.sync.dma_start(out=ld_sb[:].rearrange("p (t m) -> p t m", t=8),
                              in_=ldT[:].rearrange("(t p) m -> p t m", p=128))
            nc.sync.dma_start(out=bo_sb[:], in_=bo[:])
            nc.gpsimd.partition_broadcast(bo_bc[:], bo_sb[:])

            # PE warmup while the first activation DMAs land: keeps the HAM
            # clock-gate warm and fills the otherwise-idle load window.
            wu_ps = sum_ps.tile([1, 512], F32, tag="sums")
            for _wu in range(48):
                nc.tensor.matmul(wu_ps[:], ones[:], wq_sb[:, 0:512],
                                 start=True, stop=True)

            # ---- LoRA path: rows [c*512, (c+1)*512) of ctrl + bias ----
            with tc.tile_pool(name="ct", bufs=1) as ctp:
                ct_sb = ctp.tile([128, 8 * ROWS_PER_CORE], F32R, tag="ct")
                nc.sync.dma_start(
                    out=ct_sb[:].rearrange("p (t m) -> p t m", t=8),
                    in_=cT[:].rearrange("(t p) m -> p t m", p=128))
                dn_ps = mm_ps.tile([RANK, ROWS_PER_CORE], F32, tag="mm")
                for d in range(8):
                    nc.tensor.matmul(
                        dn_ps[:],
                        (ld_sb[:, d * RANK:(d + 1) * RANK]),
                        (ct_sb[:, d * ROWS_PER_CORE:(d + 1) * ROWS_PER_CORE]),
                        start=(d == 0), stop=(d == 7))
                dn_sb = smallp.tile([RANK, ROWS_PER_CORE], F32R, tag="dn")
                nc.vector.tensor_copy(dn_sb[:], dn_ps[:])
                for j in range(ROWS_PER_CORE // 128):
                    lo_sb = outp.tile([128, D], F32, tag="osb")
                    for h in range(2):
                        up_ps = mm_ps.tile([128, 512], F32, tag="mm")
                        nc.tensor.matmul(
                            up_ps[:],
                            (dn_sb[:, j * 128:(j + 1) * 128]),
                            (lu_sb[:, h * 512:(h + 1) * 512]),
                            start=True, stop=True)
                        nc.vector.tensor_add(
                            lo_sb[:, h * 512:(h + 1) * 512], up_ps[:],
                            bo_bc[:, h * 512:(h + 1) * 512])
                    nc.sync.dma_start(out=lora_out[j * 128:(j + 1) * 128, :],
                                      in_=lo_sb[:])

            # ---- main per-batch attention ----
            for b in range(B):
                if b not in xt_tiles:
                    load_xt(b)
                xt = xt_tiles[b]

                qt_sb = qkvp.tile([HD, S], F32R, tag="qt")
                kt_sb = qkvp.tile([HD, S], F32R, tag="kt")
                vt_sb = qkvp.tile([HD, S], F32, tag="vt", bufs=1)
                v_sb = qkvp.tile([128, S], F32R, tag="v")

                def proj_strip(w_sb, dst, strip):
                    ps = mm_ps.tile([HD, 512], F32, tag="mm", name="ps")
                    for h in range(2):
                        for dl in range(4):
                            d = h * 4 + dl
                            nc.tensor.matmul(
                                ps[:],
                                (w_sb[:, d * HD:(d + 1) * HD]),
                                (xt[h][:, dl * S + strip * 512:
                                         dl * S + strip * 512 + 512]),
                                start=(d == 0), stop=(d == 7))
                    nc.vector.tensor_copy(
                        dst[:, strip * 512:(strip + 1) * 512], ps[:])

                # V first, its transposes interleaved between Q/K strips so
                # the PE stream stays dense (transposes alone don't keep the
                # HAM clock-gate warm).
                for strip in range(NSTRIP):
                    proj_strip(wv_sb, vt_sb, strip)
                for strip in range(NSTRIP):
                    proj_strip(wq_sb, qt_sb, strip)
                    for kt in range(4 * strip, 4 * strip + 4):
                        tp = mm_ps.tile([128, 128], F32, tag="mm", name="tp")
                        nc.tensor.transpose(
                            tp[:], vt_sb[:, kt * 128:(kt + 1) * 128], ident[:])
                        nc.vector.tensor_copy(
                            v_sb[:, kt * 128:(kt + 1) * 128], tp[:])
                for strip in range(NSTRIP):
                    proj_strip(wk_sb, kt_sb, strip)

                # attention strips
                for strip in range(NSTRIP):
                    q_sl = slice(strip * 512, (strip + 1) * 512)
                    at_ps_t = at_ps.tile([HD, 512], F32, tag="at")
                    sm_ps = sum_ps.tile([1, 512], F32, tag="sums")
                    es_tiles = []
                    for kt in range(NKT):
                        scp = sc_ps.tile([128, 512], F32, tag="sc")
                        nc.tensor.matmul(
                            scp[:],
                            (kt_sb[:, kt * 128:(kt + 1) * 128]),
                            (qt_sb[:, q_sl]),
                            start=True, stop=True)
                        es = esp.tile([128, 512], F32R, tag="es")
                        es_tiles.append(es)
                        nc.scalar.activation(
                            es[:], scp[:], mybir.ActivationFunctionType.Exp,
                            scale=float(INV_SQRT_HD))
                        nc.tensor.matmul(
                            at_ps_t[:],
                            (v_sb[:, kt * 128:(kt + 1) * 128]),
                            (es[:]),
                            start=(kt == 0), stop=(kt == NKT - 1),
                            skip_group_check=True)
                        nc.tensor.matmul(
                            sm_ps[:],
                            (ones[:]),
                            (es[:]),
                            start=(kt == 0), stop=(kt == NKT - 1),
                            skip_group_check=True)

                    # sums [1,512] -> copy to SBUF, scatter to [128,4] columns,
                    # then a cheap 128-lane reciprocal (a [1,512] reciprocal
                    # runs serially on one DVE lane: ~3.3us; this way ~0.6us).
                    row_sm = smallp.tile([1, 512], F32, tag="rowsm")
                    nc.vector.tensor_copy(row_sm[:], sm_ps[:])
                    rcol_sb = smallp.tile([128, NQT], F32, tag="rcol")
                    for j in range(NQT):
                        nc.sync.dma_start(
                            out=rcol_sb[:, j:j + 1],
                            in_=row_sm[0:1, j * 128:(j + 1) * 128])
                    rc_sb = smallp.tile([128, NQT], F32, tag="rc")
                    nc.vector.reciprocal(rc_sb[:], rcol_sb[:])

                    atn_sb = smallp.tile([HD, 512], F32R, tag="atn")
                    nc.vector.tensor_copy(atn_sb[:], at_ps_t[:])

                    # output projection + normalization
                    for j in range(NQT):
                        o_sb = outp.tile([128, D], F32, tag="osb")
                        for h in range(2):
                            op = mm_ps.tile([128, 512], F32, tag="mm")
                            nc.tensor.matmul(
                                op[:],
                                (atn_sb[:, j * 128:(j + 1) * 128]),
                                (wo_sb[:, h * 512:(h + 1) * 512]),
                                start=True, stop=True)
                            nc.vector.tensor_scalar_mul(
                                o_sb[:, h * 512:(h + 1) * 512], op[:],
                                rc_sb[:, j:j + 1])
                        r0 = b * S + strip * 512 + j * 128
                        nc.sync.dma_start(out=out[r0:r0 + 128, :], in_=o_sb[:])

    nc.compile()
    _CACHE["nc"] = nc
    return nc


def _prep_in_maps(inputs):
    hidden = np.ascontiguousarray(inputs["hidden_states"], dtype=np.float32)
    control = np.ascontiguousarray(inputs["control_states"], dtype=np.float32)
    Wq = np.asarray(inputs["Wq"], dtype=np.float32)
    Wk = np.asarray(inputs["Wk"], dtype=np.float32)
    Wv = np.asarray(inputs["Wv"], dtype=np.float32)
    Wo = np.asarray(inputs["Wo"], dtype=np.float32)
    bo = np.asarray(inputs["bo"], dtype=np.float32)
    ld = np.asarray(inputs["lora_down"], dtype=np.float32)
    lu = np.asarray(inputs["lora_up"], dtype=np.float32)

    xT = np.ascontiguousarray(hidden.reshape(SG, D).T)
    cT_full = np.ascontiguousarray(control.reshape(SG, D).T)
    ldT = np.ascontiguousarray(ld.T)
    luT = np.ascontiguousarray(lu.T)
    bo_in = np.ascontiguousarray(bo.reshape(1, D))

    in_maps = []
    for c in range(N_CORES):
        hs = slice(c * HD, (c + 1) * HD)
        rs = slice(c * ROWS_PER_CORE, (c + 1) * ROWS_PER_CORE)
        in_maps.append({
            "xT": xT,
            "wqT": np.ascontiguousarray(Wq[hs, :].T),
            "wkT": np.ascontiguousarray(Wk[hs, :].T),
            "wvT": np.ascontiguousarray(Wv[hs, :].T),
            "woT": np.ascontiguousarray(Wo[:, hs].T),
            "cT": np.ascontiguousarray(cT_full[:, rs]),
            "ldT": ldT,
            "luT": luT,
            "bo": bo_in,
        })
    return in_maps


def _reduce_outputs(results):
    total = np.zeros((SG, D), dtype=np.float64)
    for c in range(N_CORES):
        total += results[c]["out"].astype(np.float64)
    total = total.astype(np.float32)
    for c in range(N_CORES):
        rs = slice(c * ROWS_PER_CORE, (c + 1) * ROWS_PER_CORE)
        total[rs] += results[c]["lora_out"]
    return total.reshape(B, S, D)


def kernel(**inputs):
    nc = build_program()
    in_maps = _prep_in_maps(inputs)
    res = run_bass_kernel_spmd(nc, in_maps, list(range(N_CORES)))
    return _reduce_outputs(res.results)
